# revision 8
# baseline (speedup 1.0000x reference)
"""CAAN kernel for Trainium2, 8-core data-parallel (one batch row per core).

Math: the reference is
    Q = R Wq^T + bq ; K = R Wk^T + bk ; V = R Wv^T + bv
    E = exp(Q K^T / sqrt(512)) ; saat = E / rowsum(E)
    winner = (saat V) W1^T W2^T + (W2 b1 + b2)

The W1/W2 head is linear, so with c = W1^T W2[0] (a 512-vector):
    winner[n] = c . (saat V)[n] + const
             = (sum_m E[n,m] * u[m]) / (sum_m E[n,m]) + const
where u = V c = R (Wv^T c) + bv.c  is a per-asset scalar. This removes the
entire V projection and attention*V matmul; only Q/K projections, the score
matrix + exp, and two partition-reductions (via a matmul against [u, ones])
remain on the device. The final divide + const runs on the host.

Per-core device work (batch row b):
  phase A: QT[d,n], KT[d,n] (bf16, biased, DVE casts) from RT = R[b]^T bf16;
           u as a 5th M=1 projection row, transposed to [128,16] via 16
           K=1 matmuls against a ones scalar.
  phase B: per 128-row m-chunk: gamma^T = K Q^T (PSUM fp32, 1024-wide
           tiles), exp -> ET bf16 (ACT), then [u_chunk|ones]^T @ ET
           accumulates s[n] (partition 0) and rowsum[n] (partition 32).
  out: s and rowsum DMA'd from PSUM to DRAM [2, 2048] f32.
"""

import math

import ml_dtypes
import numpy as np

import concourse.bass as bass
import concourse.mybir as mybir
import concourse.tile as tile
from concourse.bass_utils import run_bass_kernel_spmd
from concourse.vector_clock import ScopedClock

N_CORES = 8
NB, NN, DD = 8, 2048, 512  # batch, assets, feature dim
P = 128
NQ = DD // P   # q chunks (contraction for projections)
ND = DD // P   # d chunks
NM = NN // P   # m chunks (key/asset rows)
S = 512        # matmul moving free dim
NS = NN // S   # n slices of 512
H = 1024       # psum tile width for projections/gamma
NH = NN // H   # n slices of 1024
BF16 = mybir.dt.bfloat16
F32 = mybir.dt.float32
SCALE = 1.0 / math.sqrt(float(DD))
BF = ml_dtypes.bfloat16


class _TileContext(tile.TileContext):
    """Workaround for walrus rejecting >1 sem wait on the kernel-tail Drain
    ("Too many sync wait commands"): put each final wait on its own SP NoOp
    ahead of an unwaited Drain."""

    def _drain_and_barrier(self, tick_clock, wait_clock):
        nc = self.nc
        probe = nc.sync.nop(nofuse=True)
        wait_clock.add_sem_waits(
            probe.ins, ScopedClock({None: tick_clock.global_clock})
        )
        si = probe.ins.sync_info
        waits = list(si.on_wait) if si is not None else []
        if si is not None:
            si.on_wait = []
        for w in waits:
            n = nc.sync.nop(nofuse=True)
            n.ins.sync_info = mybir.SyncInfo(on_wait=[w], on_update=[])
        nc.sync.drain()
        nc.all_engine_barrier()
        assert self.sems is not None
        popped = nc._tile_sem_poison_stack.pop()
        assert popped is self._sem_poison
        nc.clear_and_free_semaphores(list(self.sems.allocated().values()))
        nc.all_engine_barrier()


def _split_multi_waits(nc, maxw=1):
    """This walrus build rejects instructions carrying more than one sync
    wait ("Too many sync wait commands"). Move excess waits onto same-engine
    NoOps inserted just before the instruction: sem-ge waits are monotonic
    within the kernel, so waiting for them earlier on the same engine is
    equivalent. sem-eq waits stay on the original instruction."""
    for fn in nc.m.functions:
        for blk in fn.blocks:
            insts = blk.instructions
            if not any(
                i.sync_info is not None and len(i.sync_info.on_wait) > maxw
                for i in insts
            ):
                continue
            out = []
            for inst in insts:
                si = inst.sync_info
                if si is not None and len(si.on_wait) > maxw:
                    keep = [w for w in si.on_wait if "eq" in w.wait_mode]
                    movable = [w for w in si.on_wait if "eq" not in w.wait_mode]
                    while len(keep) < maxw and movable:
                        keep.append(movable.pop(0))
                    assert len(keep) <= maxw, (
                        f"{inst.name}: {len(keep)} non-splittable waits"
                    )
                    for w in movable:
                        nop = mybir.InstNoOp(
                            name=nc.get_next_instruction_name(), ins=[], outs=[]
                        )
                        nop.engine = inst.engine
                        nop.sync_info = mybir.SyncInfo(on_wait=[w], on_update=[])
                        out.append(nop)
                    si.on_wait = keep
                out.append(inst)
            blk.instructions = out


def _build():
    nc = bass.Bass("TRN2", target_bir_lowering=False, debug=False)

    rt = nc.dram_tensor("rt", (P, NQ, NN), BF16, kind="ExternalInput")
    wqt = nc.dram_tensor("wqt", (P, NQ, DD), BF16, kind="ExternalInput")
    wkt = nc.dram_tensor("wkt", (P, NQ, DD), BF16, kind="ExternalInput")
    bqd = nc.dram_tensor("bqd", (P, ND), F32, kind="ExternalInput")
    bkd = nc.dram_tensor("bkd", (P, ND), F32, kind="ExternalInput")
    wtl = nc.dram_tensor("wtl", (P, NQ), BF16, kind="ExternalInput")
    beta = nc.dram_tensor("beta", (1, 1), F32, kind="ExternalInput")
    out = nc.dram_tensor("out", (2, NN), F32, kind="ExternalOutput")

    Ident = mybir.ActivationFunctionType.Identity
    Copy = mybir.ActivationFunctionType.Copy
    Exp = mybir.ActivationFunctionType.Exp

    with _TileContext(nc) as tc:
        with (
            tc.tile_pool(name="const", bufs=1) as cpool,
            tc.tile_pool(name="big", bufs=1) as big,
            tc.tile_pool(name="et", bufs=3) as et_pool,
        ):
            # small tensors first so lhsT weights are resident early
            wq_sb = cpool.tile([P, NQ, DD], BF16)
            nc.sync.dma_start(wq_sb[:], wqt.ap())
            wk_sb = cpool.tile([P, NQ, DD], BF16)
            nc.sync.dma_start(wk_sb[:], wkt.ap())
            bq_sb = cpool.tile([P, ND], F32)
            nc.sync.dma_start(bq_sb[:], bqd.ap())
            bk_sb = cpool.tile([P, ND], F32)
            nc.sync.dma_start(bk_sb[:], bkd.ap())
            wtl_sb = cpool.tile([P, NQ], BF16)
            nc.sync.dma_start(wtl_sb[:], wtl.ap())
            beta_sb = cpool.tile([1, 1], F32)
            nc.sync.dma_start(beta_sb[:], beta.ap())
            ones_sb = cpool.tile([1, 1], BF16)
            nc.vector.memset(ones_sb[:], 1.0)

            rt_sb = big.tile([P, NQ, NN], BF16)
            for qc in range(NQ):
                nc.sync.dma_start(rt_sb[:, qc], rt.ap()[:, qc])

            qt_sb = big.tile([P, ND, NN], BF16)
            kt_sb = big.tile([P, ND, NN], BF16)
            urow_sb = big.tile([1, NN], BF16)
            # su columns: 0 = u, 32 = ones (s lands on partition 0, rowsum
            # on partition 32 -- both legal base partitions), rest zero.
            su_sb = big.tile([P, NM, 33], BF16)
            nc.vector.memset(su_sb[:], 0.0)
            nc.vector.memset(su_sb[:, :, 32:33], 1.0)

            # ---- phase A: QT / KT projections, u row, u transpose ----
            with (
                tc.tile_pool(name="psA", bufs=2, space="PSUM") as psA,
                tc.tile_pool(name="psUr", bufs=2, space="PSUM") as psUr,
                tc.tile_pool(name="psUt", bufs=1, space="PSUM") as psUt,
            ):
                for w_sb, b_sb, o_sb in (
                    (wq_sb, bq_sb, qt_sb),
                    (wk_sb, bk_sb, kt_sb),
                ):
                    for dc in range(ND):
                        pts = [
                            psA.tile([P, H], F32, tag="proj", name=f"proj{nh}")
                            for nh in range(NH)
                        ]
                        for qc in range(NQ):
                            lhsT = w_sb[:, qc, dc * P : (dc + 1) * P]
                            for ns in range(NS):
                                nc.tensor.matmul(
                                    pts[ns // 2][:, (ns % 2) * S : (ns % 2 + 1) * S],
                                    lhsT,
                                    rt_sb[:, qc, ns * S : (ns + 1) * S],
                                    start=(qc == 0),
                                    stop=(qc == NQ - 1),
                                )
                        for nh in range(NH):
                            nc.vector.tensor_scalar_add(
                                o_sb[:, dc, nh * H : (nh + 1) * H],
                                pts[nh][:],
                                b_sb[:, dc : dc + 1],
                            )
                # u as a 5th projection row: urow[0, n] = sum_q wtl[q] RT[q, n]
                for ns in range(NS):
                    pur = psUr.tile([1, S], F32, tag="ur", name="ur")
                    for qc in range(NQ):
                        nc.tensor.matmul(
                            pur[:],
                            wtl_sb[:, qc : qc + 1],
                            rt_sb[:, qc, ns * S : (ns + 1) * S],
                            start=(qc == 0),
                            stop=(qc == NQ - 1),
                        )
                    nc.scalar.activation(
                        urow_sb[:, ns * S : (ns + 1) * S],
                        pur[:],
                        Ident,
                        bias=beta_sb[0:1, 0:1],
                        scale=1.0,
                    )
                # transpose urow [1, 2048] -> su col 0 [128, 16] via K=1 MMs
                put = psUt.tile([P, NM], F32, tag="ut", name="ut")
                for mc in range(NM):
                    nc.tensor.matmul(
                        put[:, mc : mc + 1],
                        urow_sb[0:1, mc * P : (mc + 1) * P],
                        ones_sb[0:1, 0:1],
                        start=True,
                        stop=True,
                    )
                nc.scalar.activation(su_sb[:, :, 0], put[:], Copy)

            # ---- phase B: scores, exp, s/rowsum accumulation ----
            with (
                tc.tile_pool(name="psG", bufs=2, space="PSUM") as psG,
                tc.tile_pool(name="psR", bufs=1, space="PSUM") as psR,
            ):
                srs = [
                    psR.tile([33, S], F32, tag=f"srs{ns}", name=f"srs{ns}")
                    for ns in range(NS)
                ]
                for mc in range(NM):
                    gts = [
                        psG.tile([P, H], F32, tag="g", name=f"g{nh}")
                        for nh in range(NH)
                    ]
                    for dc in range(ND):
                        lhsT = kt_sb[:, dc, mc * P : (mc + 1) * P]
                        for ns in range(NS):
                            nc.tensor.matmul(
                                gts[ns // 2][:, (ns % 2) * S : (ns % 2 + 1) * S],
                                lhsT,
                                qt_sb[:, dc, ns * S : (ns + 1) * S],
                                start=(dc == 0),
                                stop=(dc == ND - 1),
                            )
                    et = et_pool.tile([P, NN], BF16, tag="et", name="et")
                    for nh in range(NH):
                        nc.scalar.activation(
                            et[:, nh * H : (nh + 1) * H],
                            gts[nh][:],
                            Exp,
                            bias=0.0,
                            scale=SCALE,
                        )
                    for ns in range(NS):
                        nc.tensor.matmul(
                            srs[ns][:],
                            su_sb[:, mc, :],
                            et[:, ns * S : (ns + 1) * S],
                            start=(mc == 0),
                            stop=(mc == NM - 1),
                            skip_group_check=True,
                        )

                # copy PSUM -> SBUF (rows 0..32), then DMA rows 0 and 32 out
                out_sb = big.tile([33, NN], F32)
                for ns in range(NS):
                    sl = slice(ns * S, (ns + 1) * S)
                    nc.vector.tensor_copy(out_sb[:, sl], srs[ns][:])
                nc.sync.dma_start(out.ap()[0:1, :], out_sb[0:1, :])
                nc.sync.dma_start(out.ap()[1:2, :], out_sb[32:33, :])

    _split_multi_waits(nc)
    return nc


_NC = None


def _get_nc():
    global _NC
    if _NC is None:
        _NC = _build()
    return _NC


def _pack_pq(a):
    """[512, X] -> [128, 4, X] with (p, chunk) partition striping."""
    return np.ascontiguousarray(a.reshape(4, P, -1).transpose(1, 0, 2))


def kernel(R, Wq, bq, Wk, bk, Wv, bv, W1, b1, W2, b2):
    R = np.asarray(R, np.float32)
    Wq = np.asarray(Wq, np.float32)
    bq = np.asarray(bq, np.float32)
    Wk = np.asarray(Wk, np.float32)
    bk = np.asarray(bk, np.float32)
    Wv = np.asarray(Wv, np.float32)
    bv = np.asarray(bv, np.float32)
    W1 = np.asarray(W1, np.float32)
    b1 = np.asarray(b1, np.float32)
    W2 = np.asarray(W2, np.float32)
    b2 = np.asarray(b2, np.float32)

    # Collapse the linear head: winner = c.a + const, u = V c.
    c = W1.astype(np.float64).T @ W2.astype(np.float64)[0]        # [512]
    wtilde = Wv.astype(np.float64).T @ c                          # [512]
    beta = float(bv.astype(np.float64) @ c)
    const = float(
        W2.astype(np.float64)[0] @ b1.astype(np.float64)
        + b2.astype(np.float64)[0]
    )

    wq_h = _pack_pq(np.ascontiguousarray(Wq.T)).astype(BF)        # [128,4,512]
    wk_h = _pack_pq(np.ascontiguousarray(Wk.T)).astype(BF)
    bq_h = np.ascontiguousarray(bq.reshape(4, P).T)               # [128,4] f32
    bk_h = np.ascontiguousarray(bk.reshape(4, P).T)
    wtl_h = np.ascontiguousarray(wtilde.reshape(4, P).T).astype(BF)
    beta_h = np.full((1, 1), beta, np.float32)

    in_maps = []
    for b in range(NB):
        rt_h = _pack_pq(np.ascontiguousarray(R[b].T)).astype(BF)  # [128,4,2048]
        in_maps.append(
            {
                "rt": rt_h,
                "wqt": wq_h,
                "wkt": wk_h,
                "bqd": bq_h,
                "bkd": bk_h,
                "wtl": wtl_h,
                "beta": beta_h,
            }
        )

    nc = _get_nc()
    res = run_bass_kernel_spmd(nc, in_maps, core_ids=list(range(N_CORES)))
    outs = np.stack([res.results[b]["out"] for b in range(NB)])   # [8,2,2048]
    return (outs[:, 0] / outs[:, 1] + np.float32(const)).astype(np.float32)


# revision 9
# speedup vs baseline: 1.1838x; 1.1838x over previous
"""CAAN kernel for Trainium2, 8-core data-parallel (one batch row per core).

Math: the reference is
    Q = R Wq^T + bq ; K = R Wk^T + bk ; V = R Wv^T + bv
    E = exp(Q K^T / sqrt(512)) ; saat = E / rowsum(E)
    winner = (saat V) W1^T W2^T + (W2 b1 + b2)

The W1/W2 head is linear, so with c = W1^T W2[0] (a 512-vector):
    winner[n] = c . (saat V)[n] + const
             = (sum_m E[n,m] * u[m]) / (sum_m E[n,m]) + const
where u = V c = R (Wv^T c) + bv.c  is a per-asset scalar. This removes the
entire V projection and attention*V matmul; only Q/K projections, the score
matrix + exp, and two partition-reductions (via a matmul against [u, ones])
remain on the device. The final divide + const runs on the host.

Per-core device work (batch row b):
  phase A: QT[d,n], KT[d,n] (bf16, biased, DVE casts) from RT = R[b]^T bf16;
           u as an M=1 projection row, transposed to [128,16] via 16
           K=1 matmuls against a ones scalar.
  phase B: per 128-row m-chunk: gamma^T = K Q^T (PSUM fp32), exp -> ET bf16
           (ACT), then [u_chunk|ones]^T @ ET accumulates s[n] (partition 0)
           and rowsum[n] (partition 32). The s/rowsum matmuls trail one
           m-chunk behind the score matmuls so the PE never waits on exp.
  out: s and rowsum copied to SBUF and DMA'd to DRAM [2, 2048] f32.
"""

import math

import ml_dtypes
import numpy as np

import concourse.bass as bass
import concourse.mybir as mybir
import concourse.tile as tile
from concourse.bass_utils import run_bass_kernel_spmd
from concourse.vector_clock import ScopedClock

N_CORES = 8
NB, NN, DD = 8, 2048, 512  # batch, assets, feature dim
P = 128
NQ = DD // P   # q chunks (contraction for projections)
ND = DD // P   # d chunks
NM = NN // P   # m chunks (key/asset rows)
S = 512        # matmul moving free dim / PSUM bank width
NS = NN // S   # n slices of 512
BF16 = mybir.dt.bfloat16
F32 = mybir.dt.float32
SCALE = 1.0 / math.sqrt(float(DD))
BF = ml_dtypes.bfloat16


class _TileContext(tile.TileContext):
    """Workaround for walrus rejecting >1 sem wait on the kernel-tail Drain
    ("Too many sync wait commands"): put each final wait on its own SP NoOp
    ahead of an unwaited Drain."""

    def _drain_and_barrier(self, tick_clock, wait_clock):
        nc = self.nc
        probe = nc.sync.nop(nofuse=True)
        wait_clock.add_sem_waits(
            probe.ins, ScopedClock({None: tick_clock.global_clock})
        )
        si = probe.ins.sync_info
        waits = list(si.on_wait) if si is not None else []
        if si is not None:
            si.on_wait = []
        for w in waits:
            n = nc.sync.nop(nofuse=True)
            n.ins.sync_info = mybir.SyncInfo(on_wait=[w], on_update=[])
        nc.sync.drain()
        nc.all_engine_barrier()
        assert self.sems is not None
        popped = nc._tile_sem_poison_stack.pop()
        assert popped is self._sem_poison
        nc.clear_and_free_semaphores(list(self.sems.allocated().values()))
        nc.all_engine_barrier()


def _split_multi_waits(nc, maxw=1):
    """This walrus build rejects instructions carrying more than one sync
    wait ("Too many sync wait commands"). Move excess waits onto same-engine
    NoOps inserted just before the instruction: sem-ge waits are monotonic
    within the kernel, so waiting for them earlier on the same engine is
    equivalent. sem-eq waits stay on the original instruction."""
    for fn in nc.m.functions:
        for blk in fn.blocks:
            insts = blk.instructions
            if not any(
                i.sync_info is not None and len(i.sync_info.on_wait) > maxw
                for i in insts
            ):
                continue
            out = []
            for inst in insts:
                si = inst.sync_info
                if si is not None and len(si.on_wait) > maxw:
                    keep = [w for w in si.on_wait if "eq" in w.wait_mode]
                    movable = [w for w in si.on_wait if "eq" not in w.wait_mode]
                    while len(keep) < maxw and movable:
                        keep.append(movable.pop(0))
                    assert len(keep) <= maxw, (
                        f"{inst.name}: {len(keep)} non-splittable waits"
                    )
                    for w in movable:
                        nop = mybir.InstNoOp(
                            name=nc.get_next_instruction_name(), ins=[], outs=[]
                        )
                        nop.engine = inst.engine
                        nop.sync_info = mybir.SyncInfo(on_wait=[w], on_update=[])
                        out.append(nop)
                    si.on_wait = keep
                out.append(inst)
            blk.instructions = out


def _build():
    nc = bass.Bass("TRN2", target_bir_lowering=False, debug=False)

    rt = nc.dram_tensor("rt", (NQ, P, NN), BF16, kind="ExternalInput")
    wqt = nc.dram_tensor("wqt", (P, NQ, DD), BF16, kind="ExternalInput")
    wkt = nc.dram_tensor("wkt", (P, NQ, DD), BF16, kind="ExternalInput")
    bqd = nc.dram_tensor("bqd", (P, ND), F32, kind="ExternalInput")
    bkd = nc.dram_tensor("bkd", (P, ND), F32, kind="ExternalInput")
    wtl = nc.dram_tensor("wtl", (P, NQ), BF16, kind="ExternalInput")
    beta = nc.dram_tensor("beta", (1, 1), F32, kind="ExternalInput")
    out = nc.dram_tensor("out", (2, NN), F32, kind="ExternalOutput")

    Ident = mybir.ActivationFunctionType.Identity
    Copy = mybir.ActivationFunctionType.Copy
    Exp = mybir.ActivationFunctionType.Exp

    with _TileContext(nc) as tc:
        with (
            tc.tile_pool(name="const", bufs=1) as cpool,
            tc.tile_pool(name="big", bufs=1) as big,
            tc.tile_pool(name="et", bufs=3) as et_pool,
        ):
            # weights first (small, needed for lhsT); rt chunks on the
            # gpsimd (SWDGE) queues so they stream in parallel with the
            # sync-queue weight DMAs.
            wq_sb = cpool.tile([P, NQ, DD], BF16)
            nc.sync.dma_start(wq_sb[:], wqt.ap())
            rt_sb = [cpool.tile([P, NN], BF16, name=f"rt{qc}") for qc in range(NQ)]
            for qc in range(NQ):
                nc.gpsimd.dma_start(rt_sb[qc][:], rt.ap()[qc])
            bq_sb = cpool.tile([P, ND], F32)
            nc.sync.dma_start(bq_sb[:], bqd.ap())
            wk_sb = cpool.tile([P, NQ, DD], BF16)
            nc.sync.dma_start(wk_sb[:], wkt.ap())
            bk_sb = cpool.tile([P, ND], F32)
            nc.sync.dma_start(bk_sb[:], bkd.ap())
            wtl_sb = cpool.tile([P, NQ], BF16)
            nc.sync.dma_start(wtl_sb[:], wtl.ap())
            beta_sb = cpool.tile([1, 1], F32)
            nc.sync.dma_start(beta_sb[:], beta.ap())
            ones_sb = cpool.tile([1, 1], BF16)
            nc.vector.memset(ones_sb[:], 1.0)

            qt_sb = [big.tile([P, NN], BF16, name=f"qt{dc}") for dc in range(ND)]
            kt_sb = [big.tile([P, NN], BF16, name=f"kt{dc}") for dc in range(ND)]
            urow_sb = big.tile([1, NN], BF16)
            # su columns: 0 = u, 32 = ones (s lands on partition 0, rowsum
            # on partition 32 -- both legal base partitions), rest zero.
            su_sb = big.tile([P, NM, 33], BF16)
            nc.vector.memset(su_sb[:], 0.0)
            nc.vector.memset(su_sb[:, :, 32:33], 1.0)

            # ---- phase A: QT / KT projections, u row, u transpose ----
            with (
                tc.tile_pool(name="psA", bufs=4, space="PSUM") as psA,
                tc.tile_pool(name="psUr", bufs=2, space="PSUM") as psUr,
                tc.tile_pool(name="psUt", bufs=1, space="PSUM") as psUt,
            ):
                def proj(w_sb, b_sb, o_sb):
                    for dc in range(ND):
                        for ns in range(NS):
                            pt = psA.tile([P, S], F32, tag="proj", name="proj")
                            for qc in range(NQ):
                                nc.tensor.matmul(
                                    pt[:],
                                    w_sb[:, qc, dc * P : (dc + 1) * P],
                                    rt_sb[qc][:, ns * S : (ns + 1) * S],
                                    start=(qc == 0),
                                    stop=(qc == NQ - 1),
                                )
                            nc.vector.tensor_scalar_add(
                                o_sb[dc][:, ns * S : (ns + 1) * S],
                                pt[:],
                                b_sb[:, dc : dc + 1],
                            )

                proj(wq_sb, bq_sb, qt_sb)
                # u as an M=1 projection row: urow[0, n] = sum_q wtl[q] RT[q, n]
                for ns in range(NS):
                    pur = psUr.tile([1, S], F32, tag="ur", name="ur")
                    for qc in range(NQ):
                        nc.tensor.matmul(
                            pur[:],
                            wtl_sb[:, qc : qc + 1],
                            rt_sb[qc][:, ns * S : (ns + 1) * S],
                            start=(qc == 0),
                            stop=(qc == NQ - 1),
                        )
                    nc.scalar.activation(
                        urow_sb[:, ns * S : (ns + 1) * S],
                        pur[:],
                        Ident,
                        bias=beta_sb[0:1, 0:1],
                        scale=1.0,
                    )
                proj(wk_sb, bk_sb, kt_sb)
                # transpose urow [1, 2048] -> su col 0 [128, 16] via K=1 MMs
                put = psUt.tile([P, NM], F32, tag="ut", name="ut")
                for mc in range(NM):
                    nc.tensor.matmul(
                        put[:, mc : mc + 1],
                        urow_sb[0:1, mc * P : (mc + 1) * P],
                        ones_sb[0:1, 0:1],
                        start=True,
                        stop=True,
                    )
                nc.scalar.activation(su_sb[:, :, 0], put[:], Copy)

            # ---- phase B: scores, exp, s/rowsum accumulation ----
            with (
                tc.tile_pool(name="psG", bufs=4, space="PSUM") as psG,
                tc.tile_pool(name="psR", bufs=1, space="PSUM") as psR,
            ):
                srs = [
                    psR.tile([33, S], F32, tag=f"srs{ns}", name=f"srs{ns}")
                    for ns in range(NS)
                ]
                ets = {}

                def gamma(mc):
                    et = et_pool.tile([P, NN], BF16, tag="et", name="et")
                    ets[mc] = et
                    for ns in range(NS):
                        g = psG.tile([P, S], F32, tag="g", name="g")
                        for dc in range(ND):
                            nc.tensor.matmul(
                                g[:],
                                kt_sb[dc][:, mc * P : (mc + 1) * P],
                                qt_sb[dc][:, ns * S : (ns + 1) * S],
                                start=(dc == 0),
                                stop=(dc == ND - 1),
                            )
                        nc.scalar.activation(
                            et[:, ns * S : (ns + 1) * S],
                            g[:],
                            Exp,
                            bias=0.0,
                            scale=SCALE,
                        )

                def srs_mms(mc):
                    et = ets.pop(mc)
                    for ns in range(NS):
                        nc.tensor.matmul(
                            srs[ns][:],
                            su_sb[:, mc, :],
                            et[:, ns * S : (ns + 1) * S],
                            start=(mc == 0),
                            stop=(mc == NM - 1),
                            skip_group_check=True,
                        )

                # s/rowsum matmuls trail one m-chunk behind the score
                # matmuls so the PE never stalls on the exp activations.
                gamma(0)
                for mc in range(1, NM):
                    gamma(mc)
                    srs_mms(mc - 1)
                srs_mms(NM - 1)

                # copy PSUM -> SBUF (rows 0..32), then DMA rows 0 and 32 out
                out_sb = big.tile([33, NN], F32)
                for ns in range(NS):
                    sl = slice(ns * S, (ns + 1) * S)
                    nc.vector.tensor_copy(out_sb[:, sl], srs[ns][:])
                nc.sync.dma_start(out.ap()[0:1, :], out_sb[0:1, :])
                nc.sync.dma_start(out.ap()[1:2, :], out_sb[32:33, :])

    _split_multi_waits(nc)
    return nc


_NC = None


def _get_nc():
    global _NC
    if _NC is None:
        _NC = _build()
    return _NC


def _pack_pq(a):
    """[512, X] -> [128, 4, X] with (p, chunk) partition striping."""
    return np.ascontiguousarray(a.reshape(4, P, -1).transpose(1, 0, 2))


def kernel(R, Wq, bq, Wk, bk, Wv, bv, W1, b1, W2, b2):
    R = np.asarray(R, np.float32)
    Wq = np.asarray(Wq, np.float32)
    bq = np.asarray(bq, np.float32)
    Wk = np.asarray(Wk, np.float32)
    bk = np.asarray(bk, np.float32)
    Wv = np.asarray(Wv, np.float32)
    bv = np.asarray(bv, np.float32)
    W1 = np.asarray(W1, np.float32)
    b1 = np.asarray(b1, np.float32)
    W2 = np.asarray(W2, np.float32)
    b2 = np.asarray(b2, np.float32)

    # Collapse the linear head: winner = c.a + const, u = V c.
    c = W1.astype(np.float64).T @ W2.astype(np.float64)[0]        # [512]
    wtilde = Wv.astype(np.float64).T @ c                          # [512]
    beta = float(bv.astype(np.float64) @ c)
    const = float(
        W2.astype(np.float64)[0] @ b1.astype(np.float64)
        + b2.astype(np.float64)[0]
    )

    wq_h = _pack_pq(np.ascontiguousarray(Wq.T)).astype(BF)        # [128,4,512]
    wk_h = _pack_pq(np.ascontiguousarray(Wk.T)).astype(BF)
    bq_h = np.ascontiguousarray(bq.reshape(4, P).T)               # [128,4] f32
    bk_h = np.ascontiguousarray(bk.reshape(4, P).T)
    wtl_h = np.ascontiguousarray(wtilde.reshape(4, P).T).astype(BF)
    beta_h = np.full((1, 1), beta, np.float32)

    in_maps = []
    for b in range(NB):
        # [4, 128, 2048]: chunk-major so each q-chunk is one contiguous DMA
        rt_h = np.ascontiguousarray(R[b].T.reshape(4, P, NN)).astype(BF)
        in_maps.append(
            {
                "rt": rt_h,
                "wqt": wq_h,
                "wkt": wk_h,
                "bqd": bq_h,
                "bkd": bk_h,
                "wtl": wtl_h,
                "beta": beta_h,
            }
        )

    nc = _get_nc()
    res = run_bass_kernel_spmd(nc, in_maps, core_ids=list(range(N_CORES)))
    outs = np.stack([res.results[b]["out"] for b in range(NB)])   # [8,2,2048]
    return (outs[:, 0] / outs[:, 1] + np.float32(const)).astype(np.float32)


# revision 10
# speedup vs baseline: 1.2194x; 1.0301x over previous
"""CAAN kernel for Trainium2, 8-core data-parallel (one batch row per core).

Math: the reference is
    Q = R Wq^T + bq ; K = R Wk^T + bk ; V = R Wv^T + bv
    E = exp(Q K^T / sqrt(512)) ; saat = E / rowsum(E)
    winner = (saat V) W1^T W2^T + (W2 b1 + b2)

Two algebraic collapses make most of the network disappear:

1. The W1/W2 head is linear, so with c = W1^T W2[0]:
       winner[n] = (sum_m E[n,m] u[m]) / (sum_m E[n,m]) + const,
   u = V c = R (Wv^T c) + bv.c — a per-asset scalar. The V projection and
   attention*V matmul vanish.

2. gamma = Q K^T = R A R^T + (R Wq^T bk)[n] + (R Wk^T bq)[m] + bq.bk with
   A = Wq^T Wk. The per-n term scales E rows uniformly and cancels in the
   s/rowsum ratio, so it is dropped. The per-m term v[m] rides the exp
   activation's per-partition bias slot. The Q and K projections collapse
   into a single projection B = A^T-pack @ R^T.

Per-core device work (batch row b):
  phase A: B[q,m] = sum_q' A[q,q'] R[m,q'] (bf16, qc-outer waves so matmuls
           start when the first R chunk lands); u/v rows as M=1 projections,
           transposed to [128,16] columns via K=1 matmuls against a ones
           scalar.
  phase B: per 128-row m-chunk: gamma^T = B^T-slice @ R^T (PSUM fp32),
           exp(scale*psum + v) -> ET bf16 (ACT), then [u_chunk|ones]^T @ ET
           accumulates s[n] (partition 0) and rowsum[n] (partition 32).
           The s/rowsum matmuls trail one m-chunk behind the score matmuls
           so the PE never waits on exp.
  out: s and rowsum copied to SBUF, DMA'd to DRAM [2, 2048] f32; the host
       does winner = s/rowsum + const.
"""

import math

import ml_dtypes
import numpy as np

import concourse.bass as bass
import concourse.mybir as mybir
import concourse.tile as tile
from concourse.bass_utils import run_bass_kernel_spmd
from concourse.vector_clock import ScopedClock

N_CORES = 8
NB, NN, DD = 8, 2048, 512  # batch, assets, feature dim
P = 128
NQ = DD // P   # q chunks (contraction)
NM = NN // P   # m chunks (key/asset rows)
S = 512        # matmul moving free dim / PSUM bank width
NS = NN // S   # n slices of 512
BF16 = mybir.dt.bfloat16
F32 = mybir.dt.float32
SCALE = 1.0 / math.sqrt(float(DD))
BF = ml_dtypes.bfloat16


class _TileContext(tile.TileContext):
    """Workaround for walrus rejecting >1 sem wait on the kernel-tail Drain
    ("Too many sync wait commands"): put each final wait on its own SP NoOp
    ahead of an unwaited Drain."""

    def _drain_and_barrier(self, tick_clock, wait_clock):
        nc = self.nc
        probe = nc.sync.nop(nofuse=True)
        wait_clock.add_sem_waits(
            probe.ins, ScopedClock({None: tick_clock.global_clock})
        )
        si = probe.ins.sync_info
        waits = list(si.on_wait) if si is not None else []
        if si is not None:
            si.on_wait = []
        for w in waits:
            n = nc.sync.nop(nofuse=True)
            n.ins.sync_info = mybir.SyncInfo(on_wait=[w], on_update=[])
        nc.sync.drain()
        nc.all_engine_barrier()
        assert self.sems is not None
        popped = nc._tile_sem_poison_stack.pop()
        assert popped is self._sem_poison
        nc.clear_and_free_semaphores(list(self.sems.allocated().values()))
        nc.all_engine_barrier()


def _split_multi_waits(nc, maxw=1):
    """This walrus build rejects instructions carrying more than one sync
    wait ("Too many sync wait commands"). Move excess waits onto same-engine
    NoOps inserted just before the instruction: sem-ge waits are monotonic
    within the kernel, so waiting for them earlier on the same engine is
    equivalent. sem-eq waits stay on the original instruction."""
    for fn in nc.m.functions:
        for blk in fn.blocks:
            insts = blk.instructions
            if not any(
                i.sync_info is not None and len(i.sync_info.on_wait) > maxw
                for i in insts
            ):
                continue
            out = []
            for inst in insts:
                si = inst.sync_info
                if si is not None and len(si.on_wait) > maxw:
                    keep = [w for w in si.on_wait if "eq" in w.wait_mode]
                    movable = [w for w in si.on_wait if "eq" not in w.wait_mode]
                    while len(keep) < maxw and movable:
                        keep.append(movable.pop(0))
                    assert len(keep) <= maxw, (
                        f"{inst.name}: {len(keep)} non-splittable waits"
                    )
                    for w in movable:
                        nop = mybir.InstNoOp(
                            name=nc.get_next_instruction_name(), ins=[], outs=[]
                        )
                        nop.engine = inst.engine
                        nop.sync_info = mybir.SyncInfo(on_wait=[w], on_update=[])
                        out.append(nop)
                    si.on_wait = keep
                out.append(inst)
            blk.instructions = out


def _build():
    nc = bass.Bass("TRN2", target_bir_lowering=False, debug=False)

    rt = nc.dram_tensor("rt", (NQ, P, NN), BF16, kind="ExternalInput")
    amat = nc.dram_tensor("amat", (P, NQ, DD), BF16, kind="ExternalInput")
    wtl = nc.dram_tensor("wtl", (P, NQ), BF16, kind="ExternalInput")
    w2tl = nc.dram_tensor("w2tl", (P, NQ), BF16, kind="ExternalInput")
    betas = nc.dram_tensor("betas", (1, 2), F32, kind="ExternalInput")
    out = nc.dram_tensor("out", (2, NN), F32, kind="ExternalOutput")

    Ident = mybir.ActivationFunctionType.Identity
    Copy = mybir.ActivationFunctionType.Copy
    Exp = mybir.ActivationFunctionType.Exp

    with _TileContext(nc) as tc:
        with (
            tc.tile_pool(name="const", bufs=1) as cpool,
            tc.tile_pool(name="big", bufs=1) as big,
            tc.tile_pool(name="et", bufs=3) as et_pool,
        ):
            # rt chunks split across HWDGE (sync) and SWDGE (gpsimd) queues;
            # amat on sync first since the first matmul wave needs it.
            a_sb = cpool.tile([P, NQ, DD], BF16)
            nc.sync.dma_start(a_sb[:], amat.ap())
            rt_sb = [cpool.tile([P, NN], BF16, name=f"rt{qc}") for qc in range(NQ)]
            nc.gpsimd.dma_start(rt_sb[0][:], rt.ap()[0])
            nc.sync.dma_start(rt_sb[1][:], rt.ap()[1])
            nc.gpsimd.dma_start(rt_sb[2][:], rt.ap()[2])
            nc.sync.dma_start(rt_sb[3][:], rt.ap()[3])
            wtl_sb = cpool.tile([P, NQ], BF16)
            nc.gpsimd.dma_start(wtl_sb[:], wtl.ap())
            w2tl_sb = cpool.tile([P, NQ], BF16)
            nc.gpsimd.dma_start(w2tl_sb[:], w2tl.ap())
            betas_sb = cpool.tile([1, 2], F32)
            nc.gpsimd.dma_start(betas_sb[:], betas.ap())
            ones_sb = cpool.tile([1, 1], BF16)
            nc.vector.memset(ones_sb[:], 1.0)

            bt_sb = [big.tile([P, NN], BF16, name=f"bt{qc}") for qc in range(NQ)]
            urow_sb = big.tile([1, NN], BF16)
            vrow_sb = big.tile([1, NN], BF16)
            v_sb = big.tile([P, NM], F32)
            # su columns: 0 = u, 32 = ones (s lands on partition 0, rowsum
            # on partition 32 -- both legal base partitions), rest zero.
            su_sb = big.tile([P, NM, 33], BF16)
            nc.vector.memset(su_sb[:], 0.0)
            nc.vector.memset(su_sb[:, :, 32:33], 1.0)

            # ---- phase A1: B projection, qc_in-outer waves of 8 banks ----
            with tc.tile_pool(name="psA", bufs=8, space="PSUM") as psA:
                for wave in range(2):
                    qcs = (2 * wave, 2 * wave + 1)
                    pts = {
                        (qo, ns): psA.tile([P, S], F32, tag="proj", name="proj")
                        for qo in qcs
                        for ns in range(NS)
                    }
                    for qi in range(NQ):
                        for qo in qcs:
                            for ns in range(NS):
                                nc.tensor.matmul(
                                    pts[qo, ns][:],
                                    a_sb[:, qi, qo * P : (qo + 1) * P],
                                    rt_sb[qi][:, ns * S : (ns + 1) * S],
                                    start=(qi == 0),
                                    stop=(qi == NQ - 1),
                                )
                    for qo in qcs:
                        for ns in range(NS):
                            nc.vector.tensor_copy(
                                bt_sb[qo][:, ns * S : (ns + 1) * S],
                                pts[qo, ns][:],
                            )

            # ---- phase A2: u/v rows and their transposes ----
            with (
                tc.tile_pool(name="psUr", bufs=2, space="PSUM") as psUr,
                tc.tile_pool(name="psUt", bufs=2, space="PSUM") as psUt,
            ):
                for w_sb, row_sb, bidx in (
                    (wtl_sb, urow_sb, 0),
                    (w2tl_sb, vrow_sb, 1),
                ):
                    for ns in range(NS):
                        pur = psUr.tile([1, S], F32, tag="ur", name="ur")
                        for qc in range(NQ):
                            nc.tensor.matmul(
                                pur[:],
                                w_sb[:, qc : qc + 1],
                                rt_sb[qc][:, ns * S : (ns + 1) * S],
                                start=(qc == 0),
                                stop=(qc == NQ - 1),
                            )
                        nc.scalar.activation(
                            row_sb[:, ns * S : (ns + 1) * S],
                            pur[:],
                            Ident,
                            bias=betas_sb[0:1, bidx : bidx + 1],
                            scale=1.0,
                        )
                # transpose rows [1, 2048] -> columns [128, 16] via K=1 MMs
                for row_sb, tag in ((urow_sb, "ut"), (vrow_sb, "vt")):
                    put = psUt.tile([P, NM], F32, tag=tag, name=tag)
                    for mc in range(NM):
                        nc.tensor.matmul(
                            put[:, mc : mc + 1],
                            row_sb[0:1, mc * P : (mc + 1) * P],
                            ones_sb[0:1, 0:1],
                            start=True,
                            stop=True,
                        )
                    if tag == "ut":
                        nc.scalar.activation(su_sb[:, :, 0], put[:], Copy)
                    else:
                        # v lands pre-scaled so exp() can use it as bias
                        nc.scalar.activation(v_sb[:], put[:], Copy, scale=SCALE)

            # ---- phase B: scores, exp, s/rowsum accumulation ----
            with (
                tc.tile_pool(name="psG", bufs=4, space="PSUM") as psG,
                tc.tile_pool(name="psR", bufs=1, space="PSUM") as psR,
            ):
                srs = [
                    psR.tile([33, S], F32, tag=f"srs{ns}", name=f"srs{ns}")
                    for ns in range(NS)
                ]
                ets = {}

                def gamma(mc):
                    et = et_pool.tile([P, NN], BF16, tag="et", name="et")
                    ets[mc] = et
                    for ns in range(NS):
                        g = psG.tile([P, S], F32, tag="g", name="g")
                        for qc in range(NQ):
                            nc.tensor.matmul(
                                g[:],
                                bt_sb[qc][:, mc * P : (mc + 1) * P],
                                rt_sb[qc][:, ns * S : (ns + 1) * S],
                                start=(qc == 0),
                                stop=(qc == NQ - 1),
                            )
                        nc.scalar.activation(
                            et[:, ns * S : (ns + 1) * S],
                            g[:],
                            Exp,
                            bias=v_sb[:, mc : mc + 1],
                            scale=SCALE,
                        )

                def srs_mms(mc):
                    et = ets.pop(mc)
                    for ns in range(NS):
                        nc.tensor.matmul(
                            srs[ns][:],
                            su_sb[:, mc, :],
                            et[:, ns * S : (ns + 1) * S],
                            start=(mc == 0),
                            stop=(mc == NM - 1),
                            skip_group_check=True,
                        )

                # s/rowsum matmuls trail one m-chunk behind the score
                # matmuls so the PE never stalls on the exp activations.
                gamma(0)
                for mc in range(1, NM):
                    gamma(mc)
                    srs_mms(mc - 1)
                srs_mms(NM - 1)

                # copy PSUM -> SBUF (rows 0..32), then DMA rows 0 and 32 out
                out_sb = big.tile([33, NN], F32)
                for ns in range(NS):
                    sl = slice(ns * S, (ns + 1) * S)
                    nc.vector.tensor_copy(out_sb[:, sl], srs[ns][:])
                nc.sync.dma_start(out.ap()[0:1, :], out_sb[0:1, :])
                nc.sync.dma_start(out.ap()[1:2, :], out_sb[32:33, :])

    _split_multi_waits(nc)
    return nc


_NC = None


def _get_nc():
    global _NC
    if _NC is None:
        _NC = _build()
    return _NC


def _pack_pq(a):
    """[512, X] -> [128, 4, X] with (p, chunk) partition striping."""
    return np.ascontiguousarray(a.reshape(4, P, -1).transpose(1, 0, 2))


def kernel(R, Wq, bq, Wk, bk, Wv, bv, W1, b1, W2, b2):
    R = np.asarray(R, np.float32)
    Wq = np.asarray(Wq, np.float64)
    bq = np.asarray(bq, np.float64)
    Wk = np.asarray(Wk, np.float64)
    bk = np.asarray(bk, np.float64)
    Wv = np.asarray(Wv, np.float64)
    bv = np.asarray(bv, np.float64)
    W1 = np.asarray(W1, np.float64)
    b1 = np.asarray(b1, np.float64)
    W2 = np.asarray(W2, np.float64)
    b2 = np.asarray(b2, np.float64)

    # Collapse the linear head: winner = c.a + const, u = V c.
    c = W1.T @ W2[0]                      # [512]
    wtilde = Wv.T @ c                     # [512]
    beta = float(bv @ c)
    const = float(W2[0] @ b1 + b2[0])
    # Collapse the Q/K projections: gamma = R A R^T + v[m] (+ dropped n-term)
    at = Wk.T @ Wq                        # A^T = Wk^T Wq, [q', q]
    w2tilde = Wk.T @ bq                   # [512]
    beta2 = float(bq @ bk)

    a_h = _pack_pq(np.ascontiguousarray(at)).astype(BF)            # [128,4,512]
    wtl_h = np.ascontiguousarray(wtilde.reshape(4, P).T).astype(BF)
    w2tl_h = np.ascontiguousarray(w2tilde.reshape(4, P).T).astype(BF)
    betas_h = np.array([[beta, beta2]], np.float32)

    in_maps = []
    for b in range(NB):
        # [4, 128, 2048]: chunk-major so each q-chunk is one contiguous DMA
        rt_h = np.ascontiguousarray(R[b].T.reshape(4, P, NN)).astype(BF)
        in_maps.append(
            {
                "rt": rt_h,
                "amat": a_h,
                "wtl": wtl_h,
                "w2tl": w2tl_h,
                "betas": betas_h,
            }
        )

    nc = _get_nc()
    res = run_bass_kernel_spmd(nc, in_maps, core_ids=list(range(N_CORES)))
    outs = np.stack([res.results[b]["out"] for b in range(NB)])   # [8,2,2048]
    return (outs[:, 0] / outs[:, 1] + np.float32(const)).astype(np.float32)


# revision 12
# speedup vs baseline: 1.2463x; 1.0221x over previous
"""CAAN kernel for Trainium2, 8-core data-parallel (one batch row per core).

Math: the reference is
    Q = R Wq^T + bq ; K = R Wk^T + bk ; V = R Wv^T + bv
    E = exp(Q K^T / sqrt(512)) ; saat = E / rowsum(E)
    winner = (saat V) W1^T W2^T + (W2 b1 + b2)

Two algebraic collapses make most of the network disappear:

1. The W1/W2 head is linear, so with c = W1^T W2[0]:
       winner[n] = (sum_m E[n,m] u[m]) / (sum_m E[n,m]) + const,
   u = V c = R (Wv^T c) + bv.c — a per-asset scalar. The V projection and
   attention*V matmul vanish.

2. gamma = Q K^T = R A R^T + (R Wq^T bk)[n] + (R Wk^T bq)[m] + bq.bk with
   A = Wq^T Wk. The per-n term scales E rows uniformly and cancels in the
   s/rowsum ratio, so it is dropped. The per-m term v[m] rides the exp
   activation's per-partition bias slot. The Q and K projections collapse
   into a single projection B = A^T-pack @ R^T.

Per-core device work (batch row b):
  phase A: B[q,m] = sum_q' A[q,q'] R[m,q'] (bf16, qc-outer waves so matmuls
           start when the first R chunk lands); u/v rows as M=1 projections,
           transposed to [128,16] columns via K=1 matmuls against a ones
           scalar.
  phase B: per 128-row m-chunk: gamma^T = B^T-slice @ R^T (PSUM fp32),
           exp(scale*psum + v) -> ET bf16 (ACT), then [u_chunk|ones]^T @ ET
           accumulates s[n] (partition 0) and rowsum[n] (partition 32).
           The s/rowsum matmuls trail one m-chunk behind the score matmuls
           so the PE never waits on exp.
  out: s and rowsum copied to SBUF, DMA'd to DRAM [2, 2048] f32; the host
       does winner = s/rowsum + const.
"""

import math

import ml_dtypes
import numpy as np

import concourse.bass as bass
import concourse.mybir as mybir
import concourse.tile as tile
from concourse.bass_utils import run_bass_kernel_spmd
from concourse.vector_clock import ScopedClock

N_CORES = 8
NB, NN, DD = 8, 2048, 512  # batch, assets, feature dim
P = 128
NQ = DD // P   # q chunks (contraction)
NM = NN // P   # m chunks (key/asset rows)
S = 512        # matmul moving free dim / PSUM bank width
NS = NN // S   # n slices of 512
BF16 = mybir.dt.bfloat16
F32 = mybir.dt.float32
SCALE = 1.0 / math.sqrt(float(DD))
BF = ml_dtypes.bfloat16


class _TileContext(tile.TileContext):
    """Workaround for walrus rejecting >1 sem wait on the kernel-tail Drain
    ("Too many sync wait commands"): put each final wait on its own SP NoOp
    ahead of an unwaited Drain."""

    def _drain_and_barrier(self, tick_clock, wait_clock):
        nc = self.nc
        probe = nc.sync.nop(nofuse=True)
        wait_clock.add_sem_waits(
            probe.ins, ScopedClock({None: tick_clock.global_clock})
        )
        si = probe.ins.sync_info
        waits = list(si.on_wait) if si is not None else []
        if si is not None:
            si.on_wait = []
        for w in waits:
            n = nc.sync.nop(nofuse=True)
            n.ins.sync_info = mybir.SyncInfo(on_wait=[w], on_update=[])
        nc.sync.drain()
        nc.all_engine_barrier()
        assert self.sems is not None
        popped = nc._tile_sem_poison_stack.pop()
        assert popped is self._sem_poison
        nc.clear_and_free_semaphores(list(self.sems.allocated().values()))
        # the trailing all_engine_barrier is skipped: nothing after the
        # clear touches semaphores, and the runtime serializes executions


def _split_multi_waits(nc, maxw=1):
    """This walrus build rejects instructions carrying more than one sync
    wait ("Too many sync wait commands"). Move excess waits onto same-engine
    NoOps inserted just before the instruction: sem-ge waits are monotonic
    within the kernel, so waiting for them earlier on the same engine is
    equivalent. sem-eq waits stay on the original instruction."""
    for fn in nc.m.functions:
        for blk in fn.blocks:
            insts = blk.instructions
            if not any(
                i.sync_info is not None and len(i.sync_info.on_wait) > maxw
                for i in insts
            ):
                continue
            out = []
            for inst in insts:
                si = inst.sync_info
                if si is not None and len(si.on_wait) > maxw:
                    keep = [w for w in si.on_wait if "eq" in w.wait_mode]
                    movable = [w for w in si.on_wait if "eq" not in w.wait_mode]
                    while len(keep) < maxw and movable:
                        keep.append(movable.pop(0))
                    assert len(keep) <= maxw, (
                        f"{inst.name}: {len(keep)} non-splittable waits"
                    )
                    for w in movable:
                        nop = mybir.InstNoOp(
                            name=nc.get_next_instruction_name(), ins=[], outs=[]
                        )
                        nop.engine = inst.engine
                        nop.sync_info = mybir.SyncInfo(on_wait=[w], on_update=[])
                        out.append(nop)
                    si.on_wait = keep
                out.append(inst)
            blk.instructions = out


def _build():
    nc = bass.Bass("TRN2", target_bir_lowering=False, debug=False)

    rt = nc.dram_tensor("rt", (NQ, P, NN), BF16, kind="ExternalInput")
    amat = nc.dram_tensor("amat", (NQ, P, DD), BF16, kind="ExternalInput")
    wtl = nc.dram_tensor("wtl", (P, NQ), BF16, kind="ExternalInput")
    w2tl = nc.dram_tensor("w2tl", (P, NQ), BF16, kind="ExternalInput")
    betas = nc.dram_tensor("betas", (1, 2), F32, kind="ExternalInput")
    out = nc.dram_tensor("out", (2, NN), F32, kind="ExternalOutput")

    Ident = mybir.ActivationFunctionType.Identity
    Copy = mybir.ActivationFunctionType.Copy
    Exp = mybir.ActivationFunctionType.Exp

    with _TileContext(nc) as tc:
        with (
            tc.tile_pool(name="const", bufs=1) as cpool,
            tc.tile_pool(name="big", bufs=1) as big,
            tc.tile_pool(name="et", bufs=3) as et_pool,
        ):
            # rt0 + amat chunks lead on the HWDGE (sync) queue so the first
            # projection wave can start ASAP; rt2/rt3 stream on SWDGE.
            rt_sb = [cpool.tile([P, NN], BF16, name=f"rt{qc}") for qc in range(NQ)]
            a_sb = [cpool.tile([P, DD], BF16, name=f"a{qc}") for qc in range(NQ)]
            nc.sync.dma_start(rt_sb[0][:], rt.ap()[0])
            nc.sync.dma_start(a_sb[0][:], amat.ap()[0])
            wtl_sb = cpool.tile([P, NQ], BF16)
            nc.gpsimd.dma_start(wtl_sb[:], wtl.ap())
            w2tl_sb = cpool.tile([P, NQ], BF16)
            nc.gpsimd.dma_start(w2tl_sb[:], w2tl.ap())
            betas_sb = cpool.tile([1, 2], F32)
            nc.gpsimd.dma_start(betas_sb[:], betas.ap())
            nc.gpsimd.dma_start(rt_sb[2][:], rt.ap()[2])
            nc.sync.dma_start(rt_sb[1][:], rt.ap()[1])
            nc.sync.dma_start(a_sb[1][:], amat.ap()[1])
            nc.sync.dma_start(a_sb[2][:], amat.ap()[2])
            nc.sync.dma_start(a_sb[3][:], amat.ap()[3])
            nc.gpsimd.dma_start(rt_sb[3][:], rt.ap()[3])
            ones_sb = cpool.tile([1, 1], BF16)
            nc.vector.memset(ones_sb[:], 1.0)

            bt_sb = [big.tile([P, NN], BF16, name=f"bt{qc}") for qc in range(NQ)]
            urow_sb = big.tile([1, NN], BF16)
            vrow_sb = big.tile([1, NN], BF16)
            v_sb = big.tile([P, NM], F32)
            # su columns: 0 = u, 32 = ones (s lands on partition 0, rowsum
            # on partition 32 -- both legal base partitions), rest zero.
            su_sb = big.tile([P, NM, 33], BF16)
            nc.vector.memset(su_sb[:], 0.0)
            nc.vector.memset(su_sb[:, :, 32:33], 1.0)

            # ---- phase A: B projection waves with u/v rows in between ----
            def b_wave(psA, qcs):
                pts = {
                    (qo, ns): psA.tile([P, S], F32, tag="proj", name="proj")
                    for qo in qcs
                    for ns in range(NS)
                }
                for qi in range(NQ):
                    for qo in qcs:
                        for ns in range(NS):
                            nc.tensor.matmul(
                                pts[qo, ns][:],
                                a_sb[qi][:, qo * P : (qo + 1) * P],
                                rt_sb[qi][:, ns * S : (ns + 1) * S],
                                start=(qi == 0),
                                stop=(qi == NQ - 1),
                            )
                for qo in qcs:
                    for ns in range(NS):
                        nc.vector.tensor_copy(
                            bt_sb[qo][:, ns * S : (ns + 1) * S],
                            pts[qo, ns][:],
                        )

            with tc.tile_pool(name="psA", bufs=8, space="PSUM") as psA:
                b_wave(psA, (0, 1))
            with (
                tc.tile_pool(name="psUr", bufs=2, space="PSUM") as psUr,
                tc.tile_pool(name="psUt", bufs=1, space="PSUM") as psUt,
                tc.tile_pool(name="psA2", bufs=4, space="PSUM") as psA2,
            ):
                for w_sb, row_sb, bidx in (
                    (wtl_sb, urow_sb, 0),
                    (w2tl_sb, vrow_sb, 1),
                ):
                    for ns in range(NS):
                        pur = psUr.tile([1, S], F32, tag="ur", name="ur")
                        for qc in range(NQ):
                            nc.tensor.matmul(
                                pur[:],
                                w_sb[:, qc : qc + 1],
                                rt_sb[qc][:, ns * S : (ns + 1) * S],
                                start=(qc == 0),
                                stop=(qc == NQ - 1),
                            )
                        nc.scalar.activation(
                            row_sb[:, ns * S : (ns + 1) * S],
                            pur[:],
                            Ident,
                            bias=betas_sb[0:1, bidx : bidx + 1],
                            scale=1.0,
                        )
                b_wave(psA2, (2, 3))
                # transpose rows [1, 2048] -> columns [128, 16] via K=1 MMs
                for row_sb, tag in ((urow_sb, "ut"), (vrow_sb, "vt")):
                    put = psUt.tile([P, NM], F32, tag=tag, name=tag)
                    for mc in range(NM):
                        nc.tensor.matmul(
                            put[:, mc : mc + 1],
                            row_sb[0:1, mc * P : (mc + 1) * P],
                            ones_sb[0:1, 0:1],
                            start=True,
                            stop=True,
                        )
                    if tag == "ut":
                        nc.scalar.activation(su_sb[:, :, 0], put[:], Copy)
                    else:
                        # v lands pre-scaled so exp() can use it as bias
                        nc.scalar.activation(v_sb[:], put[:], Copy, scale=SCALE)

            # ---- phase B: scores, exp, s/rowsum accumulation ----
            with (
                tc.tile_pool(name="psG", bufs=4, space="PSUM") as psG,
                tc.tile_pool(name="psR", bufs=1, space="PSUM") as psR,
            ):
                srs = [
                    psR.tile([33, S], F32, tag=f"srs{ns}", name=f"srs{ns}")
                    for ns in range(NS)
                ]
                ets = {}

                def gamma(mc):
                    et = et_pool.tile([P, NN], BF16, tag="et", name="et")
                    ets[mc] = et
                    for ns in range(NS):
                        g = psG.tile([P, S], F32, tag="g", name="g")
                        for qc in range(NQ):
                            nc.tensor.matmul(
                                g[:],
                                bt_sb[qc][:, mc * P : (mc + 1) * P],
                                rt_sb[qc][:, ns * S : (ns + 1) * S],
                                start=(qc == 0),
                                stop=(qc == NQ - 1),
                            )
                        nc.scalar.activation(
                            et[:, ns * S : (ns + 1) * S],
                            g[:],
                            Exp,
                            bias=v_sb[:, mc : mc + 1],
                            scale=SCALE,
                        )

                def srs_mms(mc):
                    et = ets.pop(mc)
                    for ns in range(NS):
                        nc.tensor.matmul(
                            srs[ns][:],
                            su_sb[:, mc, :],
                            et[:, ns * S : (ns + 1) * S],
                            start=(mc == 0),
                            stop=(mc == NM - 1),
                            skip_group_check=True,
                        )

                # s/rowsum matmuls trail one m-chunk behind the score
                # matmuls so the PE never stalls on the exp activations.
                gamma(0)
                for mc in range(1, NM):
                    gamma(mc)
                    srs_mms(mc - 1)
                srs_mms(NM - 1)

                # copy PSUM -> SBUF (rows 0..32), then DMA rows 0 and 32 out
                out_sb = big.tile([33, NN], F32)
                for ns in range(NS):
                    sl = slice(ns * S, (ns + 1) * S)
                    nc.vector.tensor_copy(out_sb[:, sl], srs[ns][:])
                # (copies overlap the trailing srs matmuls per-slice)
                nc.sync.dma_start(out.ap()[0:1, :], out_sb[0:1, :])
                nc.sync.dma_start(out.ap()[1:2, :], out_sb[32:33, :])

    _split_multi_waits(nc)
    return nc


_NC = None


def _get_nc():
    global _NC
    if _NC is None:
        _NC = _build()
    return _NC


def _pack_pq(a):
    """[512, X] -> [128, 4, X] with (p, chunk) partition striping."""
    return np.ascontiguousarray(a.reshape(4, P, -1).transpose(1, 0, 2))


def kernel(R, Wq, bq, Wk, bk, Wv, bv, W1, b1, W2, b2):
    R = np.asarray(R, np.float32)
    Wq = np.asarray(Wq, np.float64)
    bq = np.asarray(bq, np.float64)
    Wk = np.asarray(Wk, np.float64)
    bk = np.asarray(bk, np.float64)
    Wv = np.asarray(Wv, np.float64)
    bv = np.asarray(bv, np.float64)
    W1 = np.asarray(W1, np.float64)
    b1 = np.asarray(b1, np.float64)
    W2 = np.asarray(W2, np.float64)
    b2 = np.asarray(b2, np.float64)

    # Collapse the linear head: winner = c.a + const, u = V c.
    c = W1.T @ W2[0]                      # [512]
    wtilde = Wv.T @ c                     # [512]
    beta = float(bv @ c)
    const = float(W2[0] @ b1 + b2[0])
    # Collapse the Q/K projections: gamma = R A R^T + v[m] (+ dropped n-term)
    at = Wk.T @ Wq                        # A^T = Wk^T Wq, [q', q]
    w2tilde = Wk.T @ bq                   # [512]
    beta2 = float(bq @ bk)

    a_h = np.ascontiguousarray(at.reshape(4, P, DD)).astype(BF)    # [4,128,512]
    wtl_h = np.ascontiguousarray(wtilde.reshape(4, P).T).astype(BF)
    w2tl_h = np.ascontiguousarray(w2tilde.reshape(4, P).T).astype(BF)
    betas_h = np.array([[beta, beta2]], np.float32)

    in_maps = []
    for b in range(NB):
        # [4, 128, 2048]: chunk-major so each q-chunk is one contiguous DMA
        rt_h = np.ascontiguousarray(R[b].T.reshape(4, P, NN)).astype(BF)
        in_maps.append(
            {
                "rt": rt_h,
                "amat": a_h,
                "wtl": wtl_h,
                "w2tl": w2tl_h,
                "betas": betas_h,
            }
        )

    nc = _get_nc()
    res = run_bass_kernel_spmd(nc, in_maps, core_ids=list(range(N_CORES)))
    outs = np.stack([res.results[b]["out"] for b in range(NB)])   # [8,2,2048]
    return (outs[:, 0] / outs[:, 1] + np.float32(const)).astype(np.float32)


# revision 13
# speedup vs baseline: 1.2655x; 1.0154x over previous
"""CAAN kernel for Trainium2, 8-core data-parallel (one batch row per core).

Math: the reference is
    Q = R Wq^T + bq ; K = R Wk^T + bk ; V = R Wv^T + bv
    E = exp(Q K^T / sqrt(512)) ; saat = E / rowsum(E)
    winner = (saat V) W1^T W2^T + (W2 b1 + b2)

Two algebraic collapses make most of the network disappear:

1. The W1/W2 head is linear, so with c = W1^T W2[0]:
       winner[n] = (sum_m E[n,m] u[m]) / (sum_m E[n,m]) + const,
   u = V c = R (Wv^T c) + bv.c — a per-asset scalar. The V projection and
   attention*V matmul vanish.

2. gamma = Q K^T = R A R^T + (R Wq^T bk)[n] + (R Wk^T bq)[m] + bq.bk with
   A = Wq^T Wk. The per-n term scales E rows uniformly and cancels in the
   s/rowsum ratio, so it is dropped. The per-m term v[m] rides the exp
   activation's per-partition bias slot. The Q and K projections collapse
   into a single projection B = A^T-pack @ R^T.

Per-core device work (batch row b):
  phase A: B[q,m] = sum_q' A[q,q'] R[m,q'] (bf16, qc-outer waves so matmuls
           start when the first R chunk lands); u/v rows as M=1 projections,
           transposed to [128,16] columns via K=1 matmuls against a ones
           scalar.
  phase B: per 128-row m-chunk: gamma^T = B^T-slice @ R^T (PSUM fp32),
           exp(scale*psum + v) -> ET bf16 (ACT), then [u_chunk|ones]^T @ ET
           accumulates s[n] (partition 0) and rowsum[n] (partition 32).
           The s/rowsum matmuls trail one m-chunk behind the score matmuls
           so the PE never waits on exp.
  out: s and rowsum copied to SBUF, DMA'd to DRAM [2, 2048] f32; the host
       does winner = s/rowsum + const.
"""

import math

import ml_dtypes
import numpy as np

import concourse.bass as bass
import concourse.mybir as mybir
import concourse.tile as tile
from concourse.bass_utils import run_bass_kernel_spmd
from concourse.vector_clock import ScopedClock

N_CORES = 8
NB, NN, DD = 8, 2048, 512  # batch, assets, feature dim
P = 128
NQ = DD // P   # q chunks (contraction)
NM = NN // P   # m chunks (key/asset rows)
S = 512        # matmul moving free dim / PSUM bank width
NS = NN // S   # n slices of 512
BF16 = mybir.dt.bfloat16
F32 = mybir.dt.float32
SCALE = 1.0 / math.sqrt(float(DD))
BF = ml_dtypes.bfloat16


class _TileContext(tile.TileContext):
    """Workaround for walrus rejecting >1 sem wait on the kernel-tail Drain
    ("Too many sync wait commands"): put each final wait on its own SP NoOp
    ahead of an unwaited Drain."""

    def _drain_and_barrier(self, tick_clock, wait_clock):
        nc = self.nc
        probe = nc.sync.nop(nofuse=True)
        wait_clock.add_sem_waits(
            probe.ins, ScopedClock({None: tick_clock.global_clock})
        )
        si = probe.ins.sync_info
        waits = list(si.on_wait) if si is not None else []
        if si is not None:
            si.on_wait = []
        for w in waits:
            n = nc.sync.nop(nofuse=True)
            n.ins.sync_info = mybir.SyncInfo(on_wait=[w], on_update=[])
        nc.sync.drain()
        nc.all_engine_barrier()
        assert self.sems is not None
        popped = nc._tile_sem_poison_stack.pop()
        assert popped is self._sem_poison
        # clear_and_free_semaphores would range-clear every ALLOCATED sem id
        # (~200+), which walrus lowers to one op per id (~7us of tail).
        # Only ids that appear in the final instruction stream can be
        # non-zero, so hardware-clear just those; do the allocator
        # bookkeeping for the full set.
        allocated = list(self.sems.allocated().values())
        sem_nums = [
            s.num if hasattr(s, "num") else int(s) for s in allocated
        ]
        used = set()
        for fn in nc.m.functions:
            for blk in fn.blocks:
                for inst in blk.instructions:
                    si = inst.sync_info
                    if si is not None:
                        for w in si.on_wait:
                            used.add(w.id)
                        for u in si.on_update:
                            used.add(u.id)
        hw_nums = sorted(n for n in sem_nums if n in used)
        for sem_range in bass.compact_to_ranges(hw_nums):
            nc.gpsimd.dma_reset(sem_range)
            nc.gpsimd.sem_clear(sem_range)
        nc._state.prepend_free_semaphores(sem_nums)
        for poison_set in nc._tile_sem_poison_stack:
            poison_set.update(sem_nums)
        # the trailing all_engine_barrier is skipped: nothing after the
        # clear touches semaphores, and the runtime serializes executions


def _split_multi_waits(nc, maxw=1):
    """This walrus build rejects instructions carrying more than one sync
    wait ("Too many sync wait commands"). Move excess waits onto same-engine
    NoOps inserted just before the instruction: sem-ge waits are monotonic
    within the kernel, so waiting for them earlier on the same engine is
    equivalent. sem-eq waits stay on the original instruction."""
    for fn in nc.m.functions:
        for blk in fn.blocks:
            insts = blk.instructions
            if not any(
                i.sync_info is not None and len(i.sync_info.on_wait) > maxw
                for i in insts
            ):
                continue
            out = []
            for inst in insts:
                si = inst.sync_info
                if si is not None and len(si.on_wait) > maxw:
                    keep = [w for w in si.on_wait if "eq" in w.wait_mode]
                    movable = [w for w in si.on_wait if "eq" not in w.wait_mode]
                    while len(keep) < maxw and movable:
                        keep.append(movable.pop(0))
                    assert len(keep) <= maxw, (
                        f"{inst.name}: {len(keep)} non-splittable waits"
                    )
                    for w in movable:
                        nop = mybir.InstNoOp(
                            name=nc.get_next_instruction_name(), ins=[], outs=[]
                        )
                        nop.engine = inst.engine
                        nop.sync_info = mybir.SyncInfo(on_wait=[w], on_update=[])
                        out.append(nop)
                    si.on_wait = keep
                out.append(inst)
            blk.instructions = out


def _build():
    nc = bass.Bass("TRN2", target_bir_lowering=False, debug=False)

    rt = nc.dram_tensor("rt", (NQ, P, NN), BF16, kind="ExternalInput")
    amat = nc.dram_tensor("amat", (NQ, P, DD), BF16, kind="ExternalInput")
    wtl = nc.dram_tensor("wtl", (P, NQ), BF16, kind="ExternalInput")
    w2tl = nc.dram_tensor("w2tl", (P, NQ), BF16, kind="ExternalInput")
    betas = nc.dram_tensor("betas", (1, 2), F32, kind="ExternalInput")
    out = nc.dram_tensor("out", (2, NN), F32, kind="ExternalOutput")

    Ident = mybir.ActivationFunctionType.Identity
    Copy = mybir.ActivationFunctionType.Copy
    Exp = mybir.ActivationFunctionType.Exp

    with _TileContext(nc) as tc:
        with (
            tc.tile_pool(name="const", bufs=1) as cpool,
            tc.tile_pool(name="big", bufs=1) as big,
            tc.tile_pool(name="et", bufs=3) as et_pool,
        ):
            # rt0 + amat chunks lead on the HWDGE (sync) queue so the first
            # projection wave can start ASAP; rt2/rt3 stream on SWDGE.
            rt_sb = [cpool.tile([P, NN], BF16, name=f"rt{qc}") for qc in range(NQ)]
            a_sb = [cpool.tile([P, DD], BF16, name=f"a{qc}") for qc in range(NQ)]
            nc.sync.dma_start(rt_sb[0][:], rt.ap()[0])
            nc.sync.dma_start(a_sb[0][:], amat.ap()[0])
            wtl_sb = cpool.tile([P, NQ], BF16)
            nc.gpsimd.dma_start(wtl_sb[:], wtl.ap())
            w2tl_sb = cpool.tile([P, NQ], BF16)
            nc.gpsimd.dma_start(w2tl_sb[:], w2tl.ap())
            betas_sb = cpool.tile([1, 2], F32)
            nc.gpsimd.dma_start(betas_sb[:], betas.ap())
            nc.gpsimd.dma_start(rt_sb[2][:], rt.ap()[2])
            nc.sync.dma_start(rt_sb[1][:], rt.ap()[1])
            nc.sync.dma_start(a_sb[1][:], amat.ap()[1])
            nc.sync.dma_start(a_sb[2][:], amat.ap()[2])
            nc.sync.dma_start(a_sb[3][:], amat.ap()[3])
            nc.gpsimd.dma_start(rt_sb[3][:], rt.ap()[3])
            ones_sb = cpool.tile([1, 1], BF16)
            nc.vector.memset(ones_sb[:], 1.0)

            bt_sb = [big.tile([P, NN], BF16, name=f"bt{qc}") for qc in range(NQ)]
            urow_sb = big.tile([1, NN], BF16)
            vrow_sb = big.tile([1, NN], BF16)
            v_sb = big.tile([P, NM], F32)
            # su columns: 0 = u, 32 = ones (s lands on partition 0, rowsum
            # on partition 32 -- both legal base partitions), rest zero.
            su_sb = big.tile([P, NM, 33], BF16)
            nc.vector.memset(su_sb[:], 0.0)
            nc.vector.memset(su_sb[:, :, 32:33], 1.0)

            # ---- phase A: B projection waves with u/v rows in between ----
            def b_wave(psA, qcs):
                pts = {
                    (qo, ns): psA.tile([P, S], F32, tag="proj", name="proj")
                    for qo in qcs
                    for ns in range(NS)
                }
                for qi in range(NQ):
                    for qo in qcs:
                        for ns in range(NS):
                            nc.tensor.matmul(
                                pts[qo, ns][:],
                                a_sb[qi][:, qo * P : (qo + 1) * P],
                                rt_sb[qi][:, ns * S : (ns + 1) * S],
                                start=(qi == 0),
                                stop=(qi == NQ - 1),
                            )
                for qo in qcs:
                    for ns in range(NS):
                        nc.vector.tensor_copy(
                            bt_sb[qo][:, ns * S : (ns + 1) * S],
                            pts[qo, ns][:],
                        )

            with tc.tile_pool(name="psA", bufs=8, space="PSUM") as psA:
                b_wave(psA, (0, 1))
            with (
                tc.tile_pool(name="psUr", bufs=2, space="PSUM") as psUr,
                tc.tile_pool(name="psUt", bufs=1, space="PSUM") as psUt,
                tc.tile_pool(name="psA2", bufs=4, space="PSUM") as psA2,
            ):
                for w_sb, row_sb, bidx in (
                    (wtl_sb, urow_sb, 0),
                    (w2tl_sb, vrow_sb, 1),
                ):
                    for ns in range(NS):
                        pur = psUr.tile([1, S], F32, tag="ur", name="ur")
                        for qc in range(NQ):
                            nc.tensor.matmul(
                                pur[:],
                                w_sb[:, qc : qc + 1],
                                rt_sb[qc][:, ns * S : (ns + 1) * S],
                                start=(qc == 0),
                                stop=(qc == NQ - 1),
                            )
                        nc.scalar.activation(
                            row_sb[:, ns * S : (ns + 1) * S],
                            pur[:],
                            Ident,
                            bias=betas_sb[0:1, bidx : bidx + 1],
                            scale=1.0,
                        )
                b_wave(psA2, (2, 3))
                # transpose rows [1, 2048] -> columns [128, 16] via K=1 MMs
                for row_sb, tag in ((urow_sb, "ut"), (vrow_sb, "vt")):
                    put = psUt.tile([P, NM], F32, tag=tag, name=tag)
                    for mc in range(NM):
                        nc.tensor.matmul(
                            put[:, mc : mc + 1],
                            row_sb[0:1, mc * P : (mc + 1) * P],
                            ones_sb[0:1, 0:1],
                            start=True,
                            stop=True,
                        )
                    if tag == "ut":
                        nc.scalar.activation(su_sb[:, :, 0], put[:], Copy)
                    else:
                        # v lands pre-scaled so exp() can use it as bias
                        nc.scalar.activation(v_sb[:], put[:], Copy, scale=SCALE)

            # ---- phase B: scores, exp, s/rowsum accumulation ----
            with (
                tc.tile_pool(name="psG", bufs=4, space="PSUM") as psG,
                tc.tile_pool(name="psR", bufs=1, space="PSUM") as psR,
            ):
                srs = [
                    psR.tile([33, S], F32, tag=f"srs{ns}", name=f"srs{ns}")
                    for ns in range(NS)
                ]
                ets = {}

                def gamma(mc):
                    et = et_pool.tile([P, NN], BF16, tag="et", name="et")
                    ets[mc] = et
                    for ns in range(NS):
                        g = psG.tile([P, S], F32, tag="g", name="g")
                        for qc in range(NQ):
                            nc.tensor.matmul(
                                g[:],
                                bt_sb[qc][:, mc * P : (mc + 1) * P],
                                rt_sb[qc][:, ns * S : (ns + 1) * S],
                                start=(qc == 0),
                                stop=(qc == NQ - 1),
                            )
                        nc.scalar.activation(
                            et[:, ns * S : (ns + 1) * S],
                            g[:],
                            Exp,
                            bias=v_sb[:, mc : mc + 1],
                            scale=SCALE,
                        )

                def srs_mms(mc):
                    et = ets.pop(mc)
                    for ns in range(NS):
                        nc.tensor.matmul(
                            srs[ns][:],
                            su_sb[:, mc, :],
                            et[:, ns * S : (ns + 1) * S],
                            start=(mc == 0),
                            stop=(mc == NM - 1),
                            skip_group_check=True,
                        )

                # s/rowsum matmuls trail one m-chunk behind the score
                # matmuls so the PE never stalls on the exp activations.
                gamma(0)
                for mc in range(1, NM):
                    gamma(mc)
                    srs_mms(mc - 1)
                srs_mms(NM - 1)

                # copy PSUM -> SBUF (rows 0..32), then DMA rows 0 and 32 out
                out_sb = big.tile([33, NN], F32)
                for ns in range(NS):
                    sl = slice(ns * S, (ns + 1) * S)
                    nc.vector.tensor_copy(out_sb[:, sl], srs[ns][:])
                # (copies overlap the trailing srs matmuls per-slice)
                nc.sync.dma_start(out.ap()[0:1, :], out_sb[0:1, :])
                nc.sync.dma_start(out.ap()[1:2, :], out_sb[32:33, :])

    _split_multi_waits(nc)
    return nc


_NC = None


def _get_nc():
    global _NC
    if _NC is None:
        _NC = _build()
    return _NC


def _pack_pq(a):
    """[512, X] -> [128, 4, X] with (p, chunk) partition striping."""
    return np.ascontiguousarray(a.reshape(4, P, -1).transpose(1, 0, 2))


def kernel(R, Wq, bq, Wk, bk, Wv, bv, W1, b1, W2, b2):
    R = np.asarray(R, np.float32)
    Wq = np.asarray(Wq, np.float64)
    bq = np.asarray(bq, np.float64)
    Wk = np.asarray(Wk, np.float64)
    bk = np.asarray(bk, np.float64)
    Wv = np.asarray(Wv, np.float64)
    bv = np.asarray(bv, np.float64)
    W1 = np.asarray(W1, np.float64)
    b1 = np.asarray(b1, np.float64)
    W2 = np.asarray(W2, np.float64)
    b2 = np.asarray(b2, np.float64)

    # Collapse the linear head: winner = c.a + const, u = V c.
    c = W1.T @ W2[0]                      # [512]
    wtilde = Wv.T @ c                     # [512]
    beta = float(bv @ c)
    const = float(W2[0] @ b1 + b2[0])
    # Collapse the Q/K projections: gamma = R A R^T + v[m] (+ dropped n-term)
    at = Wk.T @ Wq                        # A^T = Wk^T Wq, [q', q]
    w2tilde = Wk.T @ bq                   # [512]
    beta2 = float(bq @ bk)

    a_h = np.ascontiguousarray(at.reshape(4, P, DD)).astype(BF)    # [4,128,512]
    wtl_h = np.ascontiguousarray(wtilde.reshape(4, P).T).astype(BF)
    w2tl_h = np.ascontiguousarray(w2tilde.reshape(4, P).T).astype(BF)
    betas_h = np.array([[beta, beta2]], np.float32)

    in_maps = []
    for b in range(NB):
        # [4, 128, 2048]: chunk-major so each q-chunk is one contiguous DMA
        rt_h = np.ascontiguousarray(R[b].T.reshape(4, P, NN)).astype(BF)
        in_maps.append(
            {
                "rt": rt_h,
                "amat": a_h,
                "wtl": wtl_h,
                "w2tl": w2tl_h,
                "betas": betas_h,
            }
        )

    nc = _get_nc()
    res = run_bass_kernel_spmd(nc, in_maps, core_ids=list(range(N_CORES)))
    outs = np.stack([res.results[b]["out"] for b in range(NB)])   # [8,2,2048]
    return (outs[:, 0] / outs[:, 1] + np.float32(const)).astype(np.float32)


# revision 16
# speedup vs baseline: 1.2900x; 1.0193x over previous
"""CAAN kernel for Trainium2, 8-core data-parallel (one batch row per core).

Math: the reference is
    Q = R Wq^T + bq ; K = R Wk^T + bk ; V = R Wv^T + bv
    E = exp(Q K^T / sqrt(512)) ; saat = E / rowsum(E)
    winner = (saat V) W1^T W2^T + (W2 b1 + b2)

Two algebraic collapses make most of the network disappear:

1. The W1/W2 head is linear, so with c = W1^T W2[0]:
       winner[n] = (sum_m E[n,m] u[m]) / (sum_m E[n,m]) + const,
   u = V c = R (Wv^T c) + bv.c — a per-asset scalar. The V projection and
   attention*V matmul vanish.

2. gamma = Q K^T = R A R^T + (R Wq^T bk)[n] + (R Wk^T bq)[m] + bq.bk with
   A = Wq^T Wk. The per-n term scales E rows uniformly and cancels in the
   s/rowsum ratio, so it is dropped. The per-m term v[m] rides the exp
   activation's per-partition bias slot. The Q and K projections collapse
   into a single projection B = A^T-pack @ R^T.

Per-core device work (batch row b):
  phase A: B[q,m] = sum_q' A[q,q'] R[m,q'] (bf16, qc-outer waves so matmuls
           start when the first R chunk lands); u/v rows as M=1 projections,
           transposed to [128,16] columns via K=1 matmuls against a ones
           scalar.
  phase B: per 128-row m-chunk: gamma^T = B^T-slice @ R^T (PSUM fp32),
           exp(scale*psum + v) -> ET bf16 (ACT), then [u_chunk|ones]^T @ ET
           accumulates s[n] (partition 0) and rowsum[n] (partition 32).
           The s/rowsum matmuls trail one m-chunk behind the score matmuls
           so the PE never waits on exp.
  out: s and rowsum copied to SBUF, DMA'd to DRAM [2, 2048] f32; the host
       does winner = s/rowsum + const.
"""

import math

import ml_dtypes
import numpy as np

import concourse.bass as bass
import concourse.mybir as mybir
import concourse.tile as tile
from concourse.bass_utils import run_bass_kernel_spmd
from concourse.vector_clock import ScopedClock

N_CORES = 8
NB, NN, DD = 8, 2048, 512  # batch, assets, feature dim
P = 128
NQ = DD // P   # q chunks (contraction)
NM = NN // P   # m chunks (key/asset rows)
S = 512        # matmul moving free dim / PSUM bank width
NS = NN // S   # n slices of 512
BF16 = mybir.dt.bfloat16
F32 = mybir.dt.float32
SCALE = 1.0 / math.sqrt(float(DD))
BF = ml_dtypes.bfloat16


class _TileContext(tile.TileContext):
    """Workaround for walrus rejecting >1 sem wait on the kernel-tail Drain
    ("Too many sync wait commands"): put each final wait on its own SP NoOp
    ahead of an unwaited Drain."""

    def _drain_and_barrier(self, tick_clock, wait_clock):
        nc = self.nc
        probe = nc.sync.nop(nofuse=True)
        wait_clock.add_sem_waits(
            probe.ins, ScopedClock({None: tick_clock.global_clock})
        )
        si = probe.ins.sync_info
        waits = list(si.on_wait) if si is not None else []
        if si is not None:
            si.on_wait = []
        for w in waits:
            n = nc.sync.nop(nofuse=True)
            n.ins.sync_info = mybir.SyncInfo(on_wait=[w], on_update=[])
        nc.sync.drain()
        nc.all_engine_barrier()
        assert self.sems is not None
        popped = nc._tile_sem_poison_stack.pop()
        assert popped is self._sem_poison
        # clear_and_free_semaphores would range-clear every ALLOCATED sem id
        # (~200+), which walrus lowers to one op per id (~7us of tail).
        # Only ids that appear in the final instruction stream can be
        # non-zero, so hardware-clear just those; do the allocator
        # bookkeeping for the full set.
        allocated = list(self.sems.allocated().values())
        sem_nums = [
            s.num if hasattr(s, "num") else int(s) for s in allocated
        ]
        used = set()
        for fn in nc.m.functions:
            for blk in fn.blocks:
                for inst in blk.instructions:
                    si = inst.sync_info
                    if si is not None:
                        for w in si.on_wait:
                            used.add(w.id)
                        for u in si.on_update:
                            used.add(u.id)
        hw_nums = sorted(n for n in sem_nums if n in used)
        for sem_range in bass.compact_to_ranges(hw_nums):
            nc.gpsimd.dma_reset(sem_range)
            nc.gpsimd.sem_clear(sem_range)
        nc._state.prepend_free_semaphores(sem_nums)
        for poison_set in nc._tile_sem_poison_stack:
            poison_set.update(sem_nums)
        # the trailing all_engine_barrier is skipped: nothing after the
        # clear touches semaphores, and the runtime serializes executions


def _split_multi_waits(nc, maxw=1):
    """This walrus build rejects instructions carrying more than one sync
    wait ("Too many sync wait commands"). Move excess waits onto same-engine
    NoOps inserted just before the instruction: sem-ge waits are monotonic
    within the kernel, so waiting for them earlier on the same engine is
    equivalent. sem-eq waits stay on the original instruction."""
    for fn in nc.m.functions:
        for blk in fn.blocks:
            insts = blk.instructions
            if not any(
                i.sync_info is not None and len(i.sync_info.on_wait) > maxw
                for i in insts
            ):
                continue
            out = []
            for inst in insts:
                si = inst.sync_info
                if si is not None and len(si.on_wait) > maxw:
                    keep = [w for w in si.on_wait if "eq" in w.wait_mode]
                    movable = [w for w in si.on_wait if "eq" not in w.wait_mode]
                    while len(keep) < maxw and movable:
                        keep.append(movable.pop(0))
                    assert len(keep) <= maxw, (
                        f"{inst.name}: {len(keep)} non-splittable waits"
                    )
                    for w in movable:
                        nop = mybir.InstNoOp(
                            name=nc.get_next_instruction_name(), ins=[], outs=[]
                        )
                        nop.engine = inst.engine
                        nop.sync_info = mybir.SyncInfo(on_wait=[w], on_update=[])
                        out.append(nop)
                    si.on_wait = keep
                out.append(inst)
            blk.instructions = out


def _build():
    nc = bass.Bass("TRN2", target_bir_lowering=False, debug=False)

    rt = nc.dram_tensor("rt", (NQ, P, NN), BF16, kind="ExternalInput")
    amat = nc.dram_tensor("amat", (NQ, P, DD), BF16, kind="ExternalInput")
    wtl = nc.dram_tensor("wtl", (P, NQ), BF16, kind="ExternalInput")
    w2tl = nc.dram_tensor("w2tl", (P, NQ), BF16, kind="ExternalInput")
    betas = nc.dram_tensor("betas", (1, 2), F32, kind="ExternalInput")
    out = nc.dram_tensor("out", (2, NN), F32, kind="ExternalOutput")

    Ident = mybir.ActivationFunctionType.Identity
    Copy = mybir.ActivationFunctionType.Copy
    Exp = mybir.ActivationFunctionType.Exp

    with _TileContext(nc) as tc:
        with (
            tc.tile_pool(name="const", bufs=1) as cpool,
            tc.tile_pool(name="big", bufs=1) as big,
            tc.tile_pool(name="et", bufs=3) as et_pool,
            tc.tile_pool(name="dscratch", bufs=1, space="DRAM") as dpool,
        ):
            # rt0 + amat chunks lead on the HWDGE (sync) queue so the first
            # projection wave can start ASAP; rt2/rt3 stream on SWDGE.
            rt_sb = [cpool.tile([P, NN], BF16, name=f"rt{qc}") for qc in range(NQ)]
            a_sb = [cpool.tile([P, DD], BF16, name=f"a{qc}") for qc in range(NQ)]
            nc.sync.dma_start(rt_sb[0][:], rt.ap()[0])
            nc.sync.dma_start(a_sb[0][:], amat.ap()[0])
            wtl_sb = cpool.tile([P, NQ], BF16)
            nc.gpsimd.dma_start(wtl_sb[:], wtl.ap())
            w2tl_sb = cpool.tile([P, NQ], BF16)
            nc.gpsimd.dma_start(w2tl_sb[:], w2tl.ap())
            betas_sb = cpool.tile([1, 2], F32)
            nc.gpsimd.dma_start(betas_sb[:], betas.ap())
            nc.gpsimd.dma_start(rt_sb[2][:], rt.ap()[2])
            nc.sync.dma_start(rt_sb[1][:], rt.ap()[1])
            nc.sync.dma_start(a_sb[1][:], amat.ap()[1])
            nc.sync.dma_start(a_sb[2][:], amat.ap()[2])
            nc.sync.dma_start(a_sb[3][:], amat.ap()[3])
            nc.gpsimd.dma_start(rt_sb[3][:], rt.ap()[3])

            bt_sb = [big.tile([P, NN], BF16, name=f"bt{qc}") for qc in range(NQ)]
            urow_sb = big.tile([1, NN], BF16)
            vrow_sb = big.tile([1, NN], BF16)
            vcol_sb = big.tile([P, NM], BF16)
            v_sb = big.tile([P, NM], F32)
            # su columns: 0 = u, 32 = ones (s lands on partition 0, rowsum
            # on partition 32 -- both legal base partitions), rest zero.
            su_sb = big.tile([P, NM, 33], BF16)
            nc.vector.memset(su_sb[:], 0.0)
            nc.vector.memset(su_sb[:, :, 32:33], 1.0)

            # ---- phase A: B projection waves + u/v rows (one psum epoch) ----
            with (
                tc.tile_pool(name="psA", bufs=6, space="PSUM") as psA,
                tc.tile_pool(name="psUr", bufs=2, space="PSUM") as psUr,
            ):
                def b_wave(qo):
                    pts = [
                        psA.tile([P, S], F32, tag="proj", name="proj")
                        for _ in range(NS)
                    ]
                    for qi in range(NQ):
                        for ns in range(NS):
                            nc.tensor.matmul(
                                pts[ns][:],
                                a_sb[qi][:, qo * P : (qo + 1) * P],
                                rt_sb[qi][:, ns * S : (ns + 1) * S],
                                start=(qi == 0),
                                stop=(qi == NQ - 1),
                            )
                    for ns in range(NS):
                        nc.vector.tensor_copy(
                            bt_sb[qo][:, ns * S : (ns + 1) * S],
                            pts[ns][:],
                        )

                def uv_row(w_sb, row_sb, bidx, scale):
                    for ns in range(NS):
                        pur = psUr.tile([1, S], F32, tag="ur", name="ur")
                        for qc in range(NQ):
                            nc.tensor.matmul(
                                pur[:],
                                w_sb[:, qc : qc + 1],
                                rt_sb[qc][:, ns * S : (ns + 1) * S],
                                start=(qc == 0),
                                stop=(qc == NQ - 1),
                            )
                        nc.scalar.activation(
                            row_sb[:, ns * S : (ns + 1) * S],
                            pur[:],
                            Ident,
                            bias=betas_sb[0:1, bidx : bidx + 1],
                            scale=scale,
                        )

                b_wave(0)
                b_wave(1)
                uv_row(wtl_sb, urow_sb, 0, 1.0)    # u, bias beta
                b_wave(2)
                uv_row(w2tl_sb, vrow_sb, 1, SCALE)  # v, bias+scale pre-folded
                b_wave(3)

                # scatter rows [1, 2048] -> columns [128, 16] off the PE:
                # bounce through flat DRAM, where the partition-scatter read
                # pattern is expressible.
                uv_dram = dpool.tile([2, NN], BF16)
                nc.sync.dma_start(uv_dram[0:1, :], urow_sb[:])
                nc.sync.dma_start(uv_dram[1:2, :], vrow_sb[:])
                with nc.allow_non_contiguous_dma(
                    reason="2048-elem partition scatter, one-off"
                ):
                    nc.sync.dma_start(
                        su_sb[:, :, 0],
                        uv_dram[0, :].rearrange("(m p) -> p m", p=P),
                    )
                    nc.sync.dma_start(
                        vcol_sb[:],
                        uv_dram[1, :].rearrange("(m p) -> p m", p=P),
                    )
                nc.vector.tensor_copy(v_sb[:], vcol_sb[:])

            # ---- phase B: scores, exp, s/rowsum accumulation ----
            with (
                tc.tile_pool(name="psG", bufs=4, space="PSUM") as psG,
                tc.tile_pool(name="psR", bufs=1, space="PSUM") as psR,
            ):
                srs = [
                    psR.tile([33, S], F32, tag=f"srs{ns}", name=f"srs{ns}")
                    for ns in range(NS)
                ]
                ets = {}

                def gamma(mc):
                    et = et_pool.tile([P, NN], BF16, tag="et", name="et")
                    ets[mc] = et
                    for ns in range(NS):
                        g = psG.tile([P, S], F32, tag="g", name="g")
                        for qc in range(NQ):
                            nc.tensor.matmul(
                                g[:],
                                bt_sb[qc][:, mc * P : (mc + 1) * P],
                                rt_sb[qc][:, ns * S : (ns + 1) * S],
                                start=(qc == 0),
                                stop=(qc == NQ - 1),
                            )
                        nc.scalar.activation(
                            et[:, ns * S : (ns + 1) * S],
                            g[:],
                            Exp,
                            bias=v_sb[:, mc : mc + 1],
                            scale=SCALE,
                        )

                def srs_mms(mc):
                    et = ets.pop(mc)
                    for ns in range(NS):
                        nc.tensor.matmul(
                            srs[ns][:],
                            su_sb[:, mc, :],
                            et[:, ns * S : (ns + 1) * S],
                            start=(mc == 0),
                            stop=(mc == NM - 1),
                            skip_group_check=True,
                        )

                # s/rowsum matmuls trail one m-chunk behind the score
                # matmuls so the PE never stalls on the exp activations.
                gamma(0)
                for mc in range(1, NM):
                    gamma(mc)
                    srs_mms(mc - 1)
                srs_mms(NM - 1)

                # copy PSUM -> SBUF (rows 0..32), then DMA rows 0 and 32 out
                out_sb = big.tile([33, NN], F32)
                for ns in range(NS):
                    sl = slice(ns * S, (ns + 1) * S)
                    nc.vector.tensor_copy(out_sb[:, sl], srs[ns][:])
                # (copies overlap the trailing srs matmuls per-slice)
                nc.sync.dma_start(out.ap()[0:1, :], out_sb[0:1, :])
                nc.sync.dma_start(out.ap()[1:2, :], out_sb[32:33, :])

    _split_multi_waits(nc)
    return nc


_NC = None


def _get_nc():
    global _NC
    if _NC is None:
        _NC = _build()
    return _NC


def _pack_pq(a):
    """[512, X] -> [128, 4, X] with (p, chunk) partition striping."""
    return np.ascontiguousarray(a.reshape(4, P, -1).transpose(1, 0, 2))


def kernel(R, Wq, bq, Wk, bk, Wv, bv, W1, b1, W2, b2):
    R = np.asarray(R, np.float32)
    Wq = np.asarray(Wq, np.float64)
    bq = np.asarray(bq, np.float64)
    Wk = np.asarray(Wk, np.float64)
    bk = np.asarray(bk, np.float64)
    Wv = np.asarray(Wv, np.float64)
    bv = np.asarray(bv, np.float64)
    W1 = np.asarray(W1, np.float64)
    b1 = np.asarray(b1, np.float64)
    W2 = np.asarray(W2, np.float64)
    b2 = np.asarray(b2, np.float64)

    # Collapse the linear head: winner = c.a + const, u = V c.
    c = W1.T @ W2[0]                      # [512]
    wtilde = Wv.T @ c                     # [512]
    beta = float(bv @ c)
    const = float(W2[0] @ b1 + b2[0])
    # Collapse the Q/K projections: gamma = R A R^T + v[m] (+ dropped n-term)
    at = Wk.T @ Wq                        # A^T = Wk^T Wq, [q', q]
    w2tilde = Wk.T @ bq                   # [512]
    beta2 = float(bq @ bk)

    a_h = np.ascontiguousarray(at.reshape(4, P, DD)).astype(BF)    # [4,128,512]
    wtl_h = np.ascontiguousarray(wtilde.reshape(4, P).T).astype(BF)
    w2tl_h = np.ascontiguousarray(w2tilde.reshape(4, P).T).astype(BF)
    betas_h = np.array([[beta, beta2 * SCALE]], np.float32)

    in_maps = []
    for b in range(NB):
        # [4, 128, 2048]: chunk-major so each q-chunk is one contiguous DMA
        rt_h = np.ascontiguousarray(R[b].T.reshape(4, P, NN)).astype(BF)
        in_maps.append(
            {
                "rt": rt_h,
                "amat": a_h,
                "wtl": wtl_h,
                "w2tl": w2tl_h,
                "betas": betas_h,
            }
        )

    nc = _get_nc()
    res = run_bass_kernel_spmd(nc, in_maps, core_ids=list(range(N_CORES)))
    outs = np.stack([res.results[b]["out"] for b in range(NB)])   # [8,2,2048]
    return (outs[:, 0] / outs[:, 1] + np.float32(const)).astype(np.float32)
